# revision 1
# baseline (speedup 1.0000x reference)
"""MultiHeadAttention Trainium2 kernel (8 NeuronCores).

Reference computation (torch-style Linear, x @ W.T):
    k = key @ W_k.T; v = value @ W_v.T; q = query (no projection)
    scores = q @ k.T / sqrt(64) per head; attn = softmax(scores)
    out = (attn @ v) @ W_o.T

Sharding: core = (batch b, head-group g) with b in {0,1}, g in {0..3};
each core owns 4 heads of one batch. Projection weights are column-split
by head so K/V projections and attention stay core-local; the final W_o
matmul is computed as a partial sum over the core's 256 head-channels and
the 4 partials per batch are summed on host.

On-device dataflow per core (all matmuls float32r, full PE rate):
    kT[256,4096]  = W_kT.T @ keyT          (contraction over embed)
    v[4096,256]   = valueT.T @ W_vT        (+ ones column per head)
    scoresT[t,q]  = kT_h.T @ qT_h          (K=64; two heads run concurrently
                                            via tile_position rows 0/64)
    expT          = exp(scoresT / 8)       (ScalarE, from PSUM)
    outT[65,q]    = v_ext_h.T @ expT       (accumulated over 32 t-chunks;
                                            row 64 = softmax denominator)
    norm          = outT[0:64] * recip(outT[64])
    out_partial   = norm_heads.T @ W_oT    (accumulated over 4 heads)

Schedule (cost-model timeline ~350us vs PE-busy floor ~300us / ScalarE
~279us):
- Phase 1 streams K/V + projections (DMA ~93us) while TWO chase sweeps
  (q-tile 0, both head pairs) consume chunks as they are projected. The
  chase lags the stream by one chunk so ScalarE never stalls behind the
  kps->copy->scores chain, and the first wk/kblk DMAs are split so the
  first projection matmul starts ~4us in. attn@V runs in bf16 (exp output
  and projected V), same PE rate, half the SBUF.
- Phase 2 runs the remaining 6 sweeps software-pipelined with emission
  order scores(i) -> exp(i-1) -> attnv(i-2): the Tile scheduler's priority
  heap then always issues the score matmul that unblocks the next exp
  before the exp-gated attnv, keeping ScalarE at its 1038ns/chunk floor
  (measured 97.6% occupancy). W_o rides along as SINGLE-matmul micro-ops
  (one per chunk iteration, fits the ~180ns PE slack). PSUM pool creation
  order places phase-2 pools on the banks that free earliest.
- Tail: the final sweep's epilogues run in q-halves; the last q-tile's
  heads (2,3) W_o partials are DVE-added into the heads (0,1) tiles and
  stored in halves, overlapping the final DMA drain.
"""

import os
import numpy as np

import concourse.bacc as bacc
import concourse.tile as tile
import concourse.mybir as mybir
from concourse.bass_utils import run_bass_kernel_spmd

F32 = mybir.dt.float32
F32R = mybir.dt.float32r
BF16 = mybir.dt.bfloat16
EXPF = mybir.ActivationFunctionType.Exp

B, NQ, NK, E, H, D = 2, 2048, 4096, 1024, 16, 64
HPC = 4          # heads per core
C = HPC * D      # head-channels per core (256)
TB = 256         # token block for streaming K/V projections
NTB = NK // TB   # 16
TCH = NK // 128  # 32 t-chunks for attention
QT = 512         # q tile
NJ = NQ // QT    # 4

_last_results = None
_last_in_maps = None


def _build():
    nc = bacc.Bacc("TRN2", target_bir_lowering=False, debug=False, num_devices=8)

    keyT_d = nc.dram_tensor("keyT", [E, NK], F32, kind="ExternalInput").ap()
    valT_d = nc.dram_tensor("valT", [E, NK], F32, kind="ExternalInput").ap()
    qT_d = nc.dram_tensor("qT", [C, NQ], F32, kind="ExternalInput").ap()
    wkT_d = nc.dram_tensor("wkT", [E, C], F32, kind="ExternalInput").ap()
    wvT_d = nc.dram_tensor("wvT", [E, C], F32, kind="ExternalInput").ap()
    woT_d = nc.dram_tensor("woT", [D, HPC, E], F32, kind="ExternalInput").ap()
    out_d = nc.dram_tensor("out", [NQ, E], F32, kind="ExternalOutput").ap()

    keyT_r = keyT_d.rearrange("(c p) n -> p c n", p=128).bitcast(F32R)
    valT_r = valT_d.rearrange("(c p) n -> p c n", p=128).bitcast(F32R)
    qT_r = qT_d.rearrange("(c p) n -> p c n", p=128).bitcast(F32R)
    wkT_r = wkT_d.rearrange("(c p) n -> p c n", p=128).bitcast(F32R)
    wvT_r = wvT_d.rearrange("(c p) n -> p c n", p=128).bitcast(F32R)

    with tile.TileContext(nc) as tc:
        with (
            tc.tile_pool(name="wpool", bufs=1) as wpool,
            tc.tile_pool(name="stream", bufs=3) as stream,
            tc.tile_pool(name="big", bufs=1) as big,
            tc.tile_pool(name="expp", bufs=6) as expp,
            tc.tile_pool(name="epil", bufs=2) as epil,
            tc.tile_pool(name="normp", bufs=5) as normp,
            tc.tile_pool(name="outsb", bufs=4) as outsb,
        ):
            # ---- resident weights / q ----
            wk_sb = wpool.tile([128, 8, C], F32R)
            wv_sb = wpool.tile([128, 8, C], F32R)
            wo_sb = wpool.tile([D, HPC, E], F32R)
            q_sb = wpool.tile([128, 2, NQ], F32R)

            # ---- resident kT / v_ext ----
            kT_sb = big.tile([128, 2, NK], F32R)            # [hd%128, hd//128, t]
            # attn@V runs in bf16 (same PE rate as f32r, half the SBUF);
            # exp outputs and the projected V are both bf16 so the matmul is
            # dtype-homogeneous. ~0.3% rms on the attention output, far
            # inside the tolerance.
            vx_sb = big.tile([128, TCH, HPC, D + 1], BF16)  # [t%128, t//128, h, d|1]
            for t in range(TCH):
                nc.vector.memset(vx_sb[:, t, :, D:D + 1], 1.0)

            def emit_scores_pair(sdst_a, sdst_b, pr, t, q0):
                nc.tensor.matmul(sdst_a,
                                 kT_sb[0:64, pr, t * 128:(t + 1) * 128],
                                 q_sb[0:64, pr, q0:q0 + QT],
                                 start=True, stop=True, tile_position=(0, 0))
                nc.tensor.matmul(sdst_b,
                                 kT_sb[64:128, pr, t * 128:(t + 1) * 128],
                                 q_sb[64:128, pr, q0:q0 + QT],
                                 start=True, stop=True, tile_position=(64, 0))

            def emit_attnv(oA, oB, ex, pr, t):
                hA, hB = 2 * pr, 2 * pr + 1
                nc.tensor.matmul(oA[:], vx_sb[:, t, hA, :], ex[:, 0:QT],
                                 start=(t == 0), stop=(t == TCH - 1))
                nc.tensor.matmul(oB[:], vx_sb[:, t, hB, :], ex[:, QT:2 * QT],
                                 start=(t == 0), stop=(t == TCH - 1))

            def emit_epilogue(oT, norm_tiles, split=1, scalar_copy=False):
                # split>1 runs the chain on q-column slices so downstream W_o
                # groups (which consume 128-column blocks of nm) can start
                # after the first slice; scalar_copy moves the PSUM->SBUF
                # copy to the (then idle) scalar engine — both used for the
                # final, latency-critical epilogues only where DVE is the
                # serializing engine.
                ocp = epil.tile([D + 1, QT], F32, tag="ocp", bufs=3, name="ocp")
                rc = epil.tile([1, QT], F32R, tag="recip", name="rc")
                bc = epil.tile([64, QT], F32R, tag="bcast", name="bc")
                nm = normp.tile([64, QT], F32R, tag="norm", name="nm")
                w = QT // split
                for p in range(split):
                    sl = slice(p * w, (p + 1) * w)
                    if scalar_copy:
                        nc.scalar.copy(ocp[:, sl], oT[:, sl])
                    else:
                        nc.vector.tensor_copy(ocp[:, sl], oT[:, sl])
                    with nc.allow_low_precision(reason="f32r recip, ~19-bit mantissa is ample"):
                        nc.vector.reciprocal(rc[0:1, sl], ocp[64:65, sl])
                    nc.gpsimd.partition_broadcast(bc[:, sl], rc[0:1, sl])
                    nc.vector.tensor_mul(nm[:, sl], ocp[0:64, sl], bc[:, sl])
                norm_tiles.append(nm)

            # ================= PHASE 1: stream + projections + j0 =================
            norm_j0 = []
            # PSUM pool creation order fixes bank addresses: the chase
            # accumulators (freed LAST, by the j0 epilogues) sit on banks 0-3
            # so phase 2 can place pout/psw there (their first users also
            # depend on the epilogues); kps/vps and the chase score ring
            # (freed EARLY) sit on banks 4-7 where phase 2's score ring goes,
            # letting phase-2 scores+exp start before the epilogues drain.
            with (
                tc.tile_pool(name="pacc", bufs=4, space="PSUM") as pacc,
                tc.tile_pool(name="pkv", bufs=1, space="PSUM") as pkv,
                tc.tile_pool(name="ps2", bufs=2, space="PSUM") as ps2,
            ):
                # two concurrent sweeps: j=0, both pairs, single-bank score tiles
                o_acc = {}
                for pr in range(2):
                    o_acc[pr] = (pacc.tile([D + 1, QT], F32, tag="acc", name="oA"),
                                 pacc.tile([D + 1, QT], F32, tag="acc", name="oB"))

                # Critical-path DMA splitting: the first K-projection matmul
                # needs only wk[:, 0, :] and kblk[:, 0, :]. Load those two
                # small slices first so PE starts ~2us in instead of ~9us.
                kblk0 = stream.tile([128, 8, TB], F32R, tag="kblk", name="kblk0")
                nc.sync.dma_start(wk_sb[:, 0:1, :], wkT_r[:, 0:1, :])
                nc.sync.dma_start(kblk0[:, 0:1, :], keyT_r[:, 0:1, 0:TB])
                nc.sync.dma_start(wk_sb[:, 1:8, :], wkT_r[:, 1:8, :])
                nc.sync.dma_start(kblk0[:, 1:8, :], keyT_r[:, 1:8, 0:TB])
                # q's j0 slice feeds the first scores; V-side weights next.
                nc.sync.dma_start(q_sb[:, :, 0:QT], qT_r[:, :, 0:QT])
                nc.sync.dma_start(wv_sb[:], wvT_r)

                def chase_scores(t, pr, q0=0):
                    s1 = ps2.tile([128, QT], F32, tag="ssc", name="s1")
                    s2 = ps2.tile([128, QT], F32, tag="ssc", name="s2")
                    emit_scores_pair(s1[:], s2[:], pr, t, q0)
                    ex = expp.tile([128, 2 * QT], BF16, tag="exp", name="ex")
                    nc.scalar.activation(ex[:, 0:QT], s1[:], EXPF, scale=0.125)
                    nc.scalar.activation(ex[:, QT:2 * QT], s2[:], EXPF, scale=0.125)
                    return ex

                # K/V projections streamed over t blocks, interleaved with the
                # two j0 sweeps chasing the stream. Chase score/exp emissions
                # are sandwiched between projection groups so PE always has
                # the stream-critical work first in priority order.
                ex_t = {}
                for tb in range(NTB):
                    ts0 = tb * TB
                    if tb == 0:
                        kblk = kblk0
                    else:
                        kblk = stream.tile([128, 8, TB], F32R, tag="kblk", name="kblk")
                        nc.sync.dma_start(kblk[:], keyT_r[:, :, ts0:ts0 + TB])
                    vblk = stream.tile([128, 8, TB], F32R, tag="vblk", name="vblk", bufs=4)
                    nc.sync.dma_start(vblk[:], valT_r[:, :, ts0:ts0 + TB])
                    # The chase lags the stream by one chunk: during this
                    # tb's kps section (whose copies gate any NEW chunk's
                    # scores) the chase processes the PREVIOUS tb's odd chunk,
                    # so ScalarE never idles behind the projection chain.
                    kps = pkv.tile([128, 2, TB], F32, tag="kps", name="kps")
                    for mc in range(2):
                        for c in range(8):
                            nc.tensor.matmul(kps[:, mc, :], wk_sb[:, c, mc * 128:(mc + 1) * 128],
                                             kblk[:, c, :], start=(c == 0), stop=(c == 7))
                        nc.vector.tensor_copy(kT_sb[:, mc, ts0:ts0 + TB], kps[:, mc, :])
                        if tb > 0:
                            ex_t[(2 * tb - 1, mc)] = chase_scores(2 * tb - 1, mc)
                    vps = pkv.tile([128, 2, C], F32, tag="vps", name="vps")
                    for t2 in range(TB // 128):
                        for c in range(8):
                            nc.tensor.matmul(vps[:, t2, :], vblk[:, c, t2 * 128:(t2 + 1) * 128],
                                             wv_sb[:, c, :], start=(c == 0), stop=(c == 7))
                        tg = tb * (TB // 128) + t2
                        nc.vector.tensor_copy(
                            vx_sb[:, tg, :, 0:D],
                            vps[:, t2, :].rearrange("p (h d) -> p h d", h=HPC))
                        ex_t[(2 * tb, t2)] = chase_scores(2 * tb, t2)
                    for t in (2 * tb - 1, 2 * tb):
                        if t < 0:
                            continue
                        for pr in range(2):
                            emit_attnv(o_acc[pr][0], o_acc[pr][1], ex_t.pop((t, pr)), pr, t)
                # Final odd chunk (2*NTB-1) drains after the stream. Its
                # score tiles borrow the just-freed projection banks so this
                # chase pair overlaps the chunk-30 pair still in the ps2
                # ring, shortening the serial end-of-stream chain.
                tL = NK // 128 - 1
                for mc in range(2):
                    s1 = pkv.tile([128, QT], F32, tag="kps", name="s1t")
                    s2 = pkv.tile([128, QT], F32, tag="vps", name="s2t")
                    emit_scores_pair(s1[:], s2[:], mc, tL, 0)
                    exL = expp.tile([128, 2 * QT], BF16, tag="exp", name="exL")
                    nc.scalar.activation(exL[:, 0:QT], s1[:], EXPF, scale=0.125)
                    nc.scalar.activation(exL[:, QT:2 * QT], s2[:], EXPF, scale=0.125)
                    ex_t[(tL, mc)] = exL
                # q's remaining tiles + W_o are needed only in phase 2; load
                # them after the K/V stream so they don't delay projections.
                nc.sync.dma_start(q_sb[:, :, QT:NQ], qT_r[:, :, QT:NQ])
                nc.sync.dma_start(wo_sb[:], woT_d[:].bitcast(F32R))
                for pr in range(2):
                    emit_attnv(o_acc[pr][0], o_acc[pr][1],
                               ex_t.pop((NK // 128 - 1, pr)), pr, NK // 128 - 1)
                for pr in range(2):
                    emit_epilogue(o_acc[pr][0], norm_j0)
                    emit_epilogue(o_acc[pr][1], norm_j0)

            # ================= PHASE 2: j1..j3 + all W_o =================
            # Software-pipelined: per iteration emit scores(t+1), exp(t),
            # attnv(t-1) so PE always prefers the score matmul that unblocks
            # the next exp over the exp-gated attnv. The sweep stream runs
            # continuously across (j, pr) boundaries; epilogues and W_o
            # groups ride along as filler.
            with (
                # pout+psw land on the chase-accumulator banks (freed by the
                # j0 epilogues); pscore lands on the kps/vps + chase-score
                # banks which free earlier, so phase-2 scores/exp overlap the
                # phase-1 epilogue drain.
                tc.tile_pool(name="pout", bufs=3, space="PSUM") as pout,
                tc.tile_pool(name="psw", bufs=1, space="PSUM") as psw,
                tc.tile_pool(name="pscore", bufs=2, space="PSUM") as pscore,
            ):
                sweeps = [(j, pr) for j in range(1, NJ) for pr in range(2)]
                NS = len(sweeps)           # 6
                total = NS * TCH           # 192 chunk iterations

                # per-sweep state
                acc = [None] * NS          # (oA, oB)
                sc_t = [dict() for _ in range(NS)]   # live score tiles by t
                ex_t = [dict() for _ in range(NS)]   # live exp tiles by t
                norm_by_j = {0: norm_j0, 1: [], 2: [], 3: []}
                osb_by_j = {j: [None] * 4 for j in range(NJ)}

                # filler: W_o work split into SINGLE-matmul micro-ops (213ns
                # each) popped one per iteration, so PE filler fits inside
                # the per-chunk slack (~184ns) and never delays the score
                # matmul that feeds the next exp.
                filler = []

                def wo_micro_ops(j, norm_tiles, heads=tuple(range(HPC)),
                                 store=True):
                    ops = []
                    state = {}
                    for g in range(8):
                        mc, et = g // 2, g % 2

                        def make_mm(mc, et, h, first, last):
                            def fn():
                                if first:
                                    if et == 0:
                                        osb_by_j[j][mc] = outsb.tile(
                                            [128, E], F32, tag="osb", name="osb")
                                    state["wps"] = psw.tile(
                                        [128, QT], F32, tag="wps", name="wps")
                                nc.tensor.matmul(
                                    state["wps"][:],
                                    norm_tiles[h][:, mc * 128:(mc + 1) * 128],
                                    wo_sb[:, h, et * QT:(et + 1) * QT],
                                    start=first, stop=last)
                                if last:
                                    osb = osb_by_j[j][mc]
                                    nc.vector.tensor_copy(
                                        osb[:, et * QT:(et + 1) * QT], state["wps"][:])
                                    if et == 1 and store:
                                        dst = out_d[j * QT + mc * 128:
                                                    j * QT + (mc + 1) * 128, :]
                                        nc.sync.dma_start(dst, osb[:])
                            return fn

                        for idx, h in enumerate(heads):
                            ops.append(make_mm(mc, et, h, idx == 0,
                                               idx == len(heads) - 1))
                    return ops

                def sweep_of(i):
                    return i // TCH, i % TCH

                for i in range(total + 2):
                    # ---- stage A: scores for iteration i ----
                    if i < total:
                        s, t = sweep_of(i)
                        j, pr = sweeps[s]
                        if t == 0:
                            oA = pout.tile([D + 1, QT], F32, tag="outp", name="oA")
                            oB = pout.tile([D + 1, QT], F32, tag="outp", name="oB")
                            acc[s] = (oA, oB)
                        sc = pscore.tile([128, 2 * QT], F32, tag="score", name="sc")
                        emit_scores_pair(sc[:, 0:QT], sc[:, QT:2 * QT], pr, t, j * QT)
                        sc_t[s][t] = sc
                    # ---- stage B: exp for iteration i-1 ----
                    if 1 <= i <= total:
                        s, t = sweep_of(i - 1)
                        sc = sc_t[s].pop(t)
                        ex = expp.tile([128, 2 * QT], BF16, tag="exp", name="ex")
                        if s == NS - 1 and t == TCH - 1:
                            # final exp in halves so the tail's attnv and
                            # epilogue chain starts half an exp earlier
                            nc.scalar.activation(ex[:, 0:QT], sc[:, 0:QT],
                                                 EXPF, scale=0.125)
                            nc.scalar.activation(ex[:, QT:2 * QT], sc[:, QT:2 * QT],
                                                 EXPF, scale=0.125)
                        else:
                            nc.scalar.activation(ex[:], sc[:], EXPF, scale=0.125)
                        ex_t[s][t] = ex
                    # ---- stage C: attnv for iteration i-2 ----
                    if i >= 2:
                        s, t = sweep_of(i - 2)
                        j, pr = sweeps[s]
                        ex = ex_t[s].pop(t)
                        emit_attnv(acc[s][0], acc[s][1], ex, pr, t)
                        if t == TCH - 1:
                            # the final sweep's epilogues gate the W_o tail:
                            # run them in q-halves with the PSUM copy on the
                            # scalar engine so the tail starts sooner
                            last = s == NS - 1
                            emit_epilogue(acc[s][0], norm_by_j[j],
                                          split=2 if last else 1,
                                          scalar_copy=last)
                            emit_epilogue(acc[s][1], norm_by_j[j],
                                          split=2 if last else 1,
                                          scalar_copy=last)
                            acc[s] = None
                    # ---- stage D: W_o filler ----
                    # queue W_o micro-ops when a q-tile completes
                    if i >= 2:
                        s, t = sweep_of(i - 2)
                        j, pr = sweeps[s]
                        if t == TCH - 1 and pr == 1 and j < NJ - 1:
                            filler.extend(wo_micro_ops(j, norm_by_j[j]))
                        if i == 6:
                            filler.extend(wo_micro_ops(0, norm_j0))
                    # last sweep (j3, pr1): W_o for j3 heads 0,1 (available
                    # after j3/pr0's epilogue at t==1) rides along un-stored.
                    if i < total:
                        s, t = sweep_of(i)
                        if s == NS - 1 and t == 2:
                            filler.extend(wo_micro_ops(
                                NJ - 1, norm_by_j[NJ - 1], heads=(0, 1),
                                store=False))
                    # pop one micro-op per iteration; during the first
                    # sweep only every other iteration, since W_o + scores +
                    # attnv (1067ns) slightly exceeds the 1063ns exp window
                    if filler and i % 5 in (1, 3):
                        filler.pop(0)()

                # drain any remaining queued W_o groups (j0..j2)
                for fn in filler:
                    fn()
                jL = NJ - 1
                # tail: heads 2,3 of the last q-tile are summed into the
                # heads-(0,1) osb tiles on DVE (no DRAM round-trip), then each
                # mc row-block stores once. psw and the freed accumulator ring
                # alternate for 2-deep W_o pipelining.
                for g in range(8):
                    mc, et = g // 2, g % 2
                    pool, tag = ((psw, "wps") if g % 4 == 0 else (pout, "outp"))
                    wps = pool.tile([128, QT], F32, tag=tag, name="wps2")
                    for i2, h in enumerate((2, 3)):
                        nc.tensor.matmul(wps[:],
                                         norm_by_j[jL][h][:, mc * 128:(mc + 1) * 128],
                                         wo_sb[:, h, et * QT:(et + 1) * QT],
                                         start=(i2 == 0), stop=(i2 == 1))
                    osb = osb_by_j[jL][mc]
                    sl = slice(et * QT, (et + 1) * QT)
                    nc.vector.tensor_add(osb[:, sl], wps[:], osb[:, sl])
                    # store each half as soon as its add lands so the final
                    # DMA drains overlap the remaining W_o groups
                    dst = out_d[jL * QT + mc * 128:jL * QT + (mc + 1) * 128, sl]
                    nc.sync.dma_start(dst, osb[:, sl])

    nc.compile()
    return nc


_nc = None


def kernel(query, key, value, W_k, W_v, W_o):
    global _nc, _last_results, _last_in_maps
    if _nc is None:
        _nc = _build()

    query = np.asarray(query, dtype=np.float32)
    key = np.asarray(key, dtype=np.float32)
    value = np.asarray(value, dtype=np.float32)
    W_k = np.asarray(W_k, dtype=np.float32)
    W_v = np.asarray(W_v, dtype=np.float32)
    W_o = np.asarray(W_o, dtype=np.float32)

    keyT = [np.ascontiguousarray(key[b].T) for b in range(B)]
    valT = [np.ascontiguousarray(value[b].T) for b in range(B)]

    in_maps = []
    for b in range(B):
        for g in range(4):
            c0 = g * C
            woT = np.ascontiguousarray(
                W_o[:, c0:c0 + C].T.reshape(HPC, D, E).transpose(1, 0, 2))
            in_maps.append({
                "keyT": keyT[b],
                "valT": valT[b],
                "qT": np.ascontiguousarray(query[b][:, c0:c0 + C].T),
                "wkT": np.ascontiguousarray(W_k[c0:c0 + C, :].T),
                "wvT": np.ascontiguousarray(W_v[c0:c0 + C, :].T),
                "woT": woT,
            })

    _last_in_maps = in_maps
    res = run_bass_kernel_spmd(
        _nc, in_maps, core_ids=list(range(8)),
        trace=bool(os.environ.get("BASS_TRACE")))
    _last_results = res

    out = np.zeros((B, NQ, E), dtype=np.float32)
    for b in range(B):
        for g in range(4):
            out[b] += res.results[b * 4 + g]["out"]
    return out

